# revision 8
# baseline (speedup 1.0000x reference)
"""Trainium2 Bass kernel for nn_NetSoNTopSIN (topk_masking).

Computation (per full batch B=128):
  x_sun[b,c]  = mean(maps[b,c,:,:])                          [128,33]
  g           = relu(tanh(x_sun) @ w1.T)                     [128,100]
  vote[b,o,i] = g[b,i] * w2[o,i]
  x_son[s]    = sum of top-k_s of vote rows, k in TOPKS      [8,128,10]
  x_son[8]    = g @ w2.T  (== full row sums of vote)         [128,10]

Strategy: pure data parallel over batch across 8 NeuronCores (16 batches
each).  The kernel is memory-bound on streaming maps (848 MB total); the
global average pool is done with DVE free-dim reduces on [128, 6272]
tiles laid out as (batch, HW-octant) partitions, followed by a tiny
0/1-selection matmul to sum octant groups.  Top-k uses the DVE max8 +
match_replace instructions (3 passes -> top-24 sorted desc), then slice
reduces for the cumulative sums.
"""

import sys

sys.path.insert(0, "/opt/trn_rl_repo")

import numpy as np

B_LOC = 16          # batches per core
N_CORES = 8
C = 33              # channels
HW = 224 * 224      # 50176
OCT = 8             # HW octants per (b,c) row
F = HW // OCT       # 6272 elements per octant
TOPKS = [3, 4, 5, 6, 7, 10, 15, 20]
NROWS = B_LOC * 10  # 160 vote rows per core
RA = 120            # vote tile A rows (12 batches x 10)
RB = 40             # vote tile B rows (4 batches x 10)
NEG = -1e30

_CACHE = {}


def _build_program():
    import concourse.bacc as bacc
    import concourse.mybir as mybir
    from concourse.tile import TileContext

    f32 = mybir.dt.float32
    nc = bacc.Bacc("TRN2", target_bir_lowering=False, debug=False,
                   num_devices=N_CORES)

    maps = nc.dram_tensor("maps", [B_LOC, C, 224, 224], f32, kind="ExternalInput")
    w1t = nc.dram_tensor("w1t", [C, 100], f32, kind="ExternalInput")
    w2 = nc.dram_tensor("w2", [10, 100], f32, kind="ExternalInput")
    osum = nc.dram_tensor("osum", [128, B_LOC], f32, kind="ExternalInput")
    rep = nc.dram_tensor("rep", [B_LOC, NROWS], f32, kind="ExternalInput")
    t10 = nc.dram_tensor("t10", [10, NROWS], f32, kind="ExternalInput")
    ident = nc.dram_tensor("ident", [128, 128], f32, kind="ExternalInput")
    xsun_out = nc.dram_tensor("x_sun", [B_LOC, C], f32, kind="ExternalOutput")
    xson_out = nc.dram_tensor("x_son", [9, B_LOC, 10], f32, kind="ExternalOutput")

    # [b, c, h, w] -> [c][b, oct, f] with per-descriptor contiguous f
    mflat = maps.rearrange("b c h w -> b c (h w)")
    mr = mflat.rearrange("b c (o f) -> c b o f", o=OCT)

    with TileContext(nc) as tc:
        with (
            tc.tile_pool(name="consts", bufs=1) as cpool,
            tc.tile_pool(name="bigin", bufs=4) as bigpool,
            tc.tile_pool(name="small", bufs=4) as spool,
            tc.tile_pool(name="psum_small", bufs=2, space="PSUM") as pspool,
            tc.tile_pool(name="psum_xsun", bufs=1, space="PSUM") as pxpool,
        ):
            osum_sb = cpool.tile_from(osum.ap())
            rep_sb = cpool.tile_from(rep.ap())
            t10_sb = cpool.tile_from(t10.ap())
            ident_sb = cpool.tile_from(ident.ap())
            w1t_sb = cpool.tile_from(w1t.ap())
            w2_sb = cpool.tile_from(w2.ap())

            # replicated w2 rows [(b,o), i] = w2[o, i] — data-independent
            wr_sb = []
            for half, (rlo, rn) in enumerate([(0, RA), (RA, RB)]):
                wr_ps = pspool.tile([rn, 100], f32, tag="tiny")
                nc.tensor.matmul(wr_ps[:, :], lhsT=t10_sb[:, rlo:rlo + rn],
                                 rhs=w2_sb[:, :], start=True, stop=True)
                wr = cpool.tile([rn, 100], f32, tag=f"wr{half}")
                nc.vector.tensor_copy(wr[:, :], wr_ps[:, :])
                wr_sb.append(wr)

            # ---- global average pool over HW ----
            xsun_ps = pxpool.tile([B_LOC, C], f32)
            for c in range(C):
                tile_c = bigpool.tile([128, F], f32, tag="bigin")
                nc.gpsimd.dma_start(out=tile_c[:, :], in_=mr[c])
                partial = spool.tile([128, 1], f32, tag="partial")
                nc.vector.reduce_sum(out=partial[:, :], in_=tile_c[:, :],
                                     axis=mybir.AxisListType.X)
                # octant-group sums: xsun_ps[b, c] = sum_oct partial[b*8+oct]
                nc.tensor.matmul(xsun_ps[:, c:c + 1], lhsT=osum_sb[:, :],
                                 rhs=partial[:, :], start=True, stop=True)

            # ---- tiny head ----
            xs_sb = spool.tile([B_LOC, C], f32)
            nc.scalar.mul(xs_sb[:, :], xsun_ps[:, :], 1.0 / HW)
            nc.gpsimd.dma_start(out=xsun_out[:, :], in_=xs_sb[:, :])

            th = spool.tile([B_LOC, C], f32)
            nc.scalar.activation(th[:, :], xs_sb[:, :],
                                 mybir.ActivationFunctionType.Tanh)
            # transpose tanh(x_sun) -> [C, B]
            thT_ps = pspool.tile([C, B_LOC], f32, tag="tiny")
            nc.tensor.transpose(thT_ps[:, :], th[:, :], ident_sb[:B_LOC, :B_LOC])
            thT = spool.tile([C, B_LOC], f32)
            nc.vector.tensor_copy(thT[:, :], thT_ps[:, :])

            # fc1: g[b,j] = relu(sum_c thT[c,b] * w1t[c,j])
            g_ps = pspool.tile([B_LOC, 100], f32, tag="tiny")
            nc.tensor.matmul(g_ps[:, :], lhsT=thT[:, :], rhs=w1t_sb[:, :],
                             start=True, stop=True)
            g_sb = spool.tile([B_LOC, 100], f32)
            nc.scalar.activation(g_sb[:, :], g_ps[:, :],
                                 mybir.ActivationFunctionType.Relu)

            # vote rows [(b,o), i] built by two replication matmuls + DVE mul
            out9 = spool.tile([9, NROWS], f32)
            for half, (rlo, rn) in enumerate([(0, RA), (RA, RB)]):
                v_ps = pspool.tile([rn, 100], f32, tag="v")
                nc.tensor.matmul(v_ps[:, :], lhsT=rep_sb[:, rlo:rlo + rn],
                                 rhs=g_sb[:, :], start=True, stop=True)
                v0 = spool.tile([rn, 100], f32, tag=f"v0{half}")
                nc.vector.tensor_mul(v0[:, :], v_ps[:, :], wr_sb[half][:, :])

                res = spool.tile([rn, 9], f32, tag=f"res{half}")
                # dense pass = full row sums
                nc.vector.reduce_sum(out=res[:, 8:9], in_=v0[:, :],
                                     axis=mybir.AxisListType.X)
                # top-24 via 3x (max8 + match_replace)
                vals = spool.tile([rn, 24], f32, tag=f"vals{half}")
                v1 = spool.tile([rn, 100], f32, tag=f"v1{half}")
                v2 = spool.tile([rn, 100], f32, tag=f"v2{half}")
                nc.vector.max(out=vals[:, 0:8], in_=v0[:, :])
                nc.vector.match_replace(out=v1[:, :], in_to_replace=vals[:, 0:8],
                                        in_values=v0[:, :], imm_value=NEG)
                nc.vector.max(out=vals[:, 8:16], in_=v1[:, :])
                nc.vector.match_replace(out=v2[:, :], in_to_replace=vals[:, 8:16],
                                        in_values=v1[:, :], imm_value=NEG)
                nc.vector.max(out=vals[:, 16:24], in_=v2[:, :])
                for s, k in enumerate(TOPKS):
                    nc.vector.reduce_sum(out=res[:, s:s + 1], in_=vals[:, 0:k],
                                         axis=mybir.AxisListType.X)
                # transpose results -> [9, rows]
                r9_ps = pspool.tile([9, rn], f32, tag="r9")
                nc.tensor.transpose(r9_ps[:, :], res[:, :], ident_sb[:rn, :rn])
                nc.vector.tensor_copy(out9[:, rlo:rlo + rn], r9_ps[:, :])

            xson_flat = xson_out.rearrange("s b o -> s (b o)")
            nc.gpsimd.dma_start(out=xson_flat[:, :], in_=out9[:, :])

    nc.compile()
    return nc


def _consts():
    osum = np.zeros((128, B_LOC), dtype=np.float32)
    osum[np.arange(128), np.arange(128) // OCT] = 1.0
    rep = np.zeros((B_LOC, NROWS), dtype=np.float32)
    rep[np.arange(NROWS) // 10, np.arange(NROWS)] = 1.0
    t10 = np.zeros((10, NROWS), dtype=np.float32)
    t10[np.arange(NROWS) % 10, np.arange(NROWS)] = 1.0
    ident = np.eye(128, dtype=np.float32)
    return osum, rep, t10, ident


def kernel(maps, w1, w2):
    from concourse.bass_utils import run_bass_kernel_spmd

    maps = np.ascontiguousarray(np.asarray(maps), dtype=np.float32)
    w1 = np.asarray(w1, dtype=np.float32)
    w2 = np.ascontiguousarray(np.asarray(w2), dtype=np.float32)

    if "nc" not in _CACHE:
        _CACHE["nc"] = _build_program()
    nc = _CACHE["nc"]

    osum, rep, t10, ident = _consts()
    w1t = np.ascontiguousarray(w1.T)
    in_maps = []
    for i in range(N_CORES):
        in_maps.append({
            "maps": np.ascontiguousarray(maps[i * B_LOC:(i + 1) * B_LOC]),
            "w1t": w1t, "w2": w2, "osum": osum, "rep": rep, "t10": t10,
            "ident": ident,
        })
    r = run_bass_kernel_spmd(nc, in_maps, core_ids=list(range(N_CORES)))
    x_sun = np.concatenate([d["x_sun"] for d in r.results], axis=0)
    x_son = np.concatenate([d["x_son"] for d in r.results], axis=1)
    return x_sun, x_son
